# revision 1
# baseline (speedup 1.0000x reference)
"""Trainium2 Bass kernel for nn_CCFLoss (masked-MSE heat/offset losses + argmax-gathered
class-balanced BCE), data-parallel over batch across 8 NeuronCores.

Device per core (2 batches = 22 (b,c) images; processed as 11 groups of 2 images,
each group one [128, 1024] f32 tile per tensor):
  - streams 7 tensors (heat/offy/offx preds+targets, masks); DVE computes heat/offy
    masked diffs, GPSIMD computes offx's, ACT squares each masked diff with a fused
    per-partition row-sum (accum_out) into a [128, 33] accumulator.
  - per image, DVE max/max_index produce per-partition top-8 values and
    first-occurrence indices of heat_targets (exact jnp.argmax tie-break).
  - outputs per core: msum [128,33], vals8/idx8 [128, 8*22].
Host: picks the global argmax per (b,c) from per-partition top-1s, gathers clss_* at
those 176 locations, and finishes the masked means / BCE on scalars in float64.
"""
import sys

if "/opt/trn_rl_repo" not in sys.path:
    sys.path.insert(0, "/opt/trn_rl_repo")

import numpy as np

B, C, H, W = 16, 11, 256, 256
P, F = 128, 512            # one [H,W] image = [128, 512] tile
T = 2                      # images per group tile
NCORES = 8
BPC = B // NCORES          # batches per core
NPAIR = BPC * C            # images per core (22)
NGROUP = NPAIR // T        # group tiles per core (11)
N_V_CHANNELS = 5
HEAT_WEIGHT = 1.0
OFFSET_WEIGHT = 1.0

_IN_NAMES = ("ht", "oxp", "oxt", "hp", "m", "oyp", "oyt")

_STATE = {}


def _pos_weight(samples):
    s = np.asarray(samples, dtype=np.float64)
    beta = (s - 1.0) / s
    en = (1.0 - np.power(beta, s)) / (1.0 - beta)
    w = 1.0 / (en + 1e-5)
    return float(w[1] / (w[0] + 1e-5))


POS_W_V = _pos_weight([8000.0, 2000.0])
POS_W_D = _pos_weight([7000.0, 2000.0 + 1000.0])


def _build():
    import concourse.bacc as bacc
    import concourse.tile as tile
    import concourse.mybir as mybir

    f32 = mybir.dt.float32
    bf16 = mybir.dt.bfloat16
    u32 = mybir.dt.uint32
    SQUARE = mybir.ActivationFunctionType.Square

    nc = bacc.Bacc("TRN2", target_bir_lowering=False, debug=False)
    # host supplies [NGROUP, P, T*F]: each group is one fully contiguous 512KB
    # block, partition-major, so a group DMA is a single sequential HBM read
    ins = {
        name: nc.dram_tensor(name, [NPAIR // T, P, T * F], f32,
                             kind="ExternalInput").ap()
        for name in _IN_NAMES
    }
    msum_d = nc.dram_tensor("msum", [P, 3 * NGROUP], f32, kind="ExternalOutput").ap()
    vals_d = nc.dram_tensor("vals8", [P, 8 * NPAIR], f32, kind="ExternalOutput").ap()
    idx_d = nc.dram_tensor("idx8", [P, 8 * NPAIR], u32, kind="ExternalOutput").ap()

    FT = F * T

    with tile.TileContext(nc) as tc:
        with tc.tile_pool(name="ins", bufs=4) as ipool, \
             tc.tile_pool(name="work", bufs=3) as wpool, \
             tc.tile_pool(name="acc", bufs=1) as apool:
            msum_t = apool.tile([P, 3 * NGROUP], f32)
            vals_t = apool.tile([P, 8 * NPAIR], f32)
            idx_t = apool.tile([P, 8 * NPAIR], u32)

            for g in range(NGROUP):
                t = {}
                for j, name in enumerate(_IN_NAMES):
                    tt = ipool.tile([P, FT], f32, tag=name)
                    # first group: co-issue from ACT so transfers start on
                    # whichever sequencer boots first
                    eng = nc.scalar if (g == 0 and j % 2 == 1) else nc.sync
                    eng.dma_start(out=tt[:], in_=ins[name][g])
                    t[name] = tt

                # masks cast to bf16 on ACT so the products run in DVE 2x bf16 mode
                mb = wpool.tile([P, FT], bf16, tag="mb")
                nc.scalar.copy(out=mb[:], in_=t["m"][:])
                htb = wpool.tile([P, FT], bf16, tag="htb")
                nc.scalar.copy(out=htb[:], in_=t["ht"][:])

                # heat: sum((hp - ht) * m)^2           (DVE diff, ACT square+rowsum)
                dh = wpool.tile([P, FT], bf16, tag="dh")
                nc.vector.tensor_sub(out=dh[:], in0=t["hp"][:], in1=t["ht"][:])
                dhm = wpool.tile([P, FT], bf16, tag="dhm")
                nc.vector.tensor_mul(out=dhm[:], in0=dh[:], in1=mb[:])
                sq = wpool.tile([P, FT], bf16, tag="sq")
                nc.scalar.activation(sq[:], dhm[:], SQUARE,
                                     accum_out=msum_t[:, 3 * g:3 * g + 1])

                # offx: sum((oxp - oxt) * ht)^2        (diffs on GPSIMD to offload DVE)
                dx = wpool.tile([P, FT], bf16, tag="dx")
                nc.gpsimd.tensor_sub(out=dx[:], in0=t["oxp"][:], in1=t["oxt"][:])
                dxm = wpool.tile([P, FT], bf16, tag="dxm")
                nc.gpsimd.tensor_mul(out=dxm[:], in0=dx[:], in1=htb[:])
                sq3 = wpool.tile([P, FT], bf16, tag="sq")
                nc.scalar.activation(sq3[:], dxm[:], SQUARE,
                                     accum_out=msum_t[:, 3 * g + 2:3 * g + 3])

                # per-image per-partition top-8 values + first-occurrence indices
                for k in range(T):
                    i = T * g + k
                    img = t["ht"][:, k * F:(k + 1) * F]
                    v8 = vals_t[:, 8 * i:8 * i + 8]
                    nc.vector.max(out=v8, in_=img)
                    nc.vector.max_index(out=idx_t[:, 8 * i:8 * i + 8],
                                        in_max=v8, in_values=img)

                # offy: sum((oyp - oyt) * ht)^2
                dy = wpool.tile([P, FT], bf16, tag="dy")
                nc.vector.tensor_sub(out=dy[:], in0=t["oyp"][:], in1=t["oyt"][:])
                dym = wpool.tile([P, FT], bf16, tag="dym")
                nc.vector.tensor_mul(out=dym[:], in0=dy[:], in1=htb[:])
                sq2 = wpool.tile([P, FT], bf16, tag="sq")
                nc.scalar.activation(sq2[:], dym[:], SQUARE,
                                     accum_out=msum_t[:, 3 * g + 1:3 * g + 2])


            nc.sync.dma_start(out=msum_d, in_=msum_t[:])
            nc.sync.dma_start(out=vals_d, in_=vals_t[:])
            nc.sync.dma_start(out=idx_d, in_=idx_t[:])

    nc.compile()
    return nc


def _get_nc():
    if "nc" not in _STATE:
        _STATE["nc"] = _build()
    return _STATE["nc"]


def _softplus(x):
    return np.log1p(np.exp(-np.abs(x))) + np.maximum(x, 0.0)


def run_device(in_maps, **kwargs):
    from concourse.bass_utils import run_bass_kernel_spmd
    nc = _get_nc()
    return run_bass_kernel_spmd(nc, in_maps, core_ids=list(range(NCORES)), **kwargs)


def make_in_maps(inp):
    hp = np.ascontiguousarray(inp["heat_predictions"], dtype=np.float32)
    ht = np.ascontiguousarray(inp["heat_targets"], dtype=np.float32)
    m = np.ascontiguousarray(inp["masks"], dtype=np.float32)
    oyp = np.ascontiguousarray(inp["offy_predictions"], dtype=np.float32)
    oyt = np.ascontiguousarray(inp["offy_targets"], dtype=np.float32)
    oxp = np.ascontiguousarray(inp["offx_predictions"], dtype=np.float32)
    oxt = np.ascontiguousarray(inp["offx_targets"], dtype=np.float32)
    full = {"hp": hp, "ht": ht, "m": m, "oyp": oyp, "oyt": oyt,
            "oxp": oxp, "oxt": oxt}
    in_maps = []
    for k in range(NCORES):
        im = {name: np.ascontiguousarray(
                  arr[k * BPC:(k + 1) * BPC].reshape(NPAIR // T, T, P, F)
                  .transpose(0, 2, 1, 3).reshape(NPAIR // T, P, T * F))
              for name, arr in full.items()}
        in_maps.append(im)
    return in_maps


def finish_host(results, inp):
    """Combine per-core device outputs into the final scalar loss (float64 host math)."""
    cp = np.asarray(inp["clss_predictions"], dtype=np.float32).reshape(B, C, H * W)
    ct = np.asarray(inp["clss_targets"], dtype=np.float32).reshape(B, C, H * W)
    v_w = float(np.asarray(inp["v_loss_weight"]))
    d_w = float(np.asarray(inp["d_loss_weight"]))

    ssq = np.zeros(3, dtype=np.float64)
    g_pred = np.zeros((B, C), dtype=np.float64)
    g_tgt = np.zeros((B, C), dtype=np.float64)
    for k in range(NCORES):
        out = results[k]
        mm = np.asarray(out["msum"], dtype=np.float64).reshape(P, NGROUP, 3)
        ssq += mm.sum(axis=(0, 1))
        pm = np.asarray(out["vals8"]).reshape(P, NPAIR, 8)[:, :, 0]
        ji = np.asarray(out["idx8"]).reshape(P, NPAIR, 8)[:, :, 0]
        for i in range(NPAIR):
            b = k * BPC + i // C
            c = i % C
            p_star = int(np.argmax(pm[:, i]))  # first max partition == lowest flat idx
            flat = p_star * F + int(ji[p_star, i])
            g_pred[b, c] = cp[b, c, flat]
            g_tgt[b, c] = ct[b, c, flat]

    n_el = float(B * C * H * W)
    heat_loss = ssq[0] / n_el
    offy_loss = ssq[1] / n_el
    offx_loss = ssq[2] / n_el

    valid = g_tgt >= 0.0
    is_v = (np.arange(C) < N_V_CHANNELS)[None, :]
    v_mask = (valid & is_v).astype(np.float64)
    d_mask = (valid & ~is_v).astype(np.float64)

    x = g_pred
    sp_neg = _softplus(-x)
    sp_pos = _softplus(x)

    l_v = POS_W_V * g_tgt * sp_neg + (1.0 - g_tgt) * sp_pos
    v_cls = (l_v * v_mask).sum() / max(v_mask.sum(), 1.0)
    y_d = (g_tgt >= 1.0).astype(np.float64)
    l_d = POS_W_D * y_d * sp_neg + (1.0 - y_d) * sp_pos
    d_cls = (l_d * d_mask).sum() / max(d_mask.sum(), 1.0)

    loss = (heat_loss * HEAT_WEIGHT
            + offy_loss * OFFSET_WEIGHT + offx_loss * OFFSET_WEIGHT
            + v_cls * v_w + d_cls * d_w)
    return np.float32(loss)


def kernel(**inputs):
    inp = {k: np.asarray(v) for k, v in inputs.items()}
    in_maps = make_in_maps(inp)
    res = run_device(in_maps)
    return finish_host(res.results, inp)



# revision 3
# speedup vs baseline: 1.0441x; 1.0441x over previous
"""Trainium2 Bass kernel for nn_CCFLoss (masked-MSE heat/offset losses + argmax-gathered
class-balanced BCE), data-parallel over batch across 8 NeuronCores.

v2: inputs stream as bf16 (except heat_targets, kept f32 for the exact argmax
tie-break), cutting HBM traffic from 28 B/elem to 16 B/elem. Each image is a
[64, 1024] block; groups stack 2 images on the partition axis -> [128, 1024]
tiles, so one max8/max_index pair covers both images.

Device per core (2 batches = 22 images; 11 groups of 2):
  - DVE: heat sub/mul + offy sub/mul (bf16 2x mode), max8 + max_index on f32 ht.
  - GPSIMD: offx sub/mul.
  - ACT: ht f32->bf16 cast + 3x Square with fused per-partition row-sum accum.
  - outputs per core: msum [128, 33], vals8 [128, 88] f32, idx8 [128, 88] u32.
Host: picks the global argmax per (b,c) from per-partition top-1s, gathers clss_*
at those 176 locations, finishes the masked means / BCE on scalars in float64.
"""
import sys

if "/opt/trn_rl_repo" not in sys.path:
    sys.path.insert(0, "/opt/trn_rl_repo")

import numpy as np
import ml_dtypes

B, C, H, W = 16, 11, 256, 256
P = 128
PI, FI = 64, 1024          # one [H,W] image = [64, 1024] block
T = 2                      # images per group tile (stacked on partitions)
NCORES = 8
BPC = B // NCORES          # batches per core
NPAIR = BPC * C            # images per core (22)
NGROUP = NPAIR // T        # group tiles per core (11)
N_V_CHANNELS = 5
HEAT_WEIGHT = 1.0
OFFSET_WEIGHT = 1.0

_BF16_NAMES = ("hp", "m", "oyp", "oyt", "oxp", "oxt")

_STATE = {}


def _pos_weight(samples):
    s = np.asarray(samples, dtype=np.float64)
    beta = (s - 1.0) / s
    en = (1.0 - np.power(beta, s)) / (1.0 - beta)
    w = 1.0 / (en + 1e-5)
    return float(w[1] / (w[0] + 1e-5))


POS_W_V = _pos_weight([8000.0, 2000.0])
POS_W_D = _pos_weight([7000.0, 2000.0 + 1000.0])


def _build():
    import concourse.bacc as bacc
    import concourse.tile as tile
    import concourse.mybir as mybir

    f32 = mybir.dt.float32
    bf16 = mybir.dt.bfloat16
    u32 = mybir.dt.uint32
    SQUARE = mybir.ActivationFunctionType.Square

    nc = bacc.Bacc("TRN2", target_bir_lowering=False, debug=False)
    # host supplies [NGROUP, P, FI]: each group one contiguous block (2 images
    # back to back), so a group DMA is a single sequential HBM read
    ins = {"ht": nc.dram_tensor("ht", [NGROUP, P, FI], f32, kind="ExternalInput").ap()}
    for name in _BF16_NAMES:
        ins[name] = nc.dram_tensor(name, [NGROUP, P, FI], bf16,
                                   kind="ExternalInput").ap()
    msum_d = nc.dram_tensor("msum", [P, 3 * NGROUP], f32, kind="ExternalOutput").ap()
    vals_d = nc.dram_tensor("vals8", [P, 8 * NGROUP], f32, kind="ExternalOutput").ap()
    idx_d = nc.dram_tensor("idx8", [P, 8 * NGROUP], u32, kind="ExternalOutput").ap()

    with tile.TileContext(nc) as tc:
        with tc.tile_pool(name="ins", bufs=4) as ipool, \
             tc.tile_pool(name="work", bufs=3) as wpool, \
             tc.tile_pool(name="acc", bufs=1) as apool:
            msum_t = apool.tile([P, 3 * NGROUP], f32)
            vals_t = apool.tile([P, 8 * NGROUP], f32)
            idx_t = apool.tile([P, 8 * NGROUP], u32)

            for g in range(NGROUP):
                t = {}
                for j, name in enumerate(("ht",) + _BF16_NAMES):
                    dt = f32 if name == "ht" else bf16
                    tt = ipool.tile([P, FI], dt, tag=name)
                    # first group: co-issue from ACT so transfers start on
                    # whichever sequencer boots first
                    eng = nc.scalar if (g == 0 and j % 2 == 1) else nc.sync
                    eng.dma_start(out=tt[:], in_=ins[name][g])
                    t[name] = tt

                # per-partition top-8 of ht (f32, exact) — covers both images
                v8 = vals_t[:, 8 * g:8 * g + 8]
                nc.vector.max(out=v8, in_=t["ht"][:])
                nc.vector.max_index(out=idx_t[:, 8 * g:8 * g + 8],
                                    in_max=v8, in_values=t["ht"][:])

                # ht cast for the bf16 elementwise chain
                htb = wpool.tile([P, FI], bf16, tag="htb")
                nc.scalar.copy(out=htb[:], in_=t["ht"][:])

                # heat: sum((hp - ht) * m)^2         (DVE diffs, ACT square+rowsum)
                dh = wpool.tile([P, FI], bf16, tag="dh")
                nc.vector.tensor_sub(out=dh[:], in0=t["hp"][:], in1=htb[:])
                dhm = wpool.tile([P, FI], bf16, tag="dhm")
                nc.vector.tensor_mul(out=dhm[:], in0=dh[:], in1=t["m"][:])
                sq = wpool.tile([P, FI], bf16, tag="sq")
                nc.scalar.activation(sq[:], dhm[:], SQUARE,
                                     accum_out=msum_t[:, 3 * g:3 * g + 1])

                # offy: sum((oyp - oyt) * ht)^2
                dy = wpool.tile([P, FI], bf16, tag="dy")
                nc.vector.tensor_sub(out=dy[:], in0=t["oyp"][:], in1=t["oyt"][:])
                dym = wpool.tile([P, FI], bf16, tag="dym")
                nc.vector.tensor_mul(out=dym[:], in0=dy[:], in1=htb[:])
                sq2 = wpool.tile([P, FI], bf16, tag="sq")
                nc.scalar.activation(sq2[:], dym[:], SQUARE,
                                     accum_out=msum_t[:, 3 * g + 1:3 * g + 2])

                # offx: sum((oxp - oxt) * ht)^2       (on GPSIMD to offload DVE)
                dx = wpool.tile([P, FI], bf16, tag="dx")
                nc.gpsimd.tensor_sub(out=dx[:], in0=t["oxp"][:], in1=t["oxt"][:])
                dxm = wpool.tile([P, FI], bf16, tag="dxm")
                nc.gpsimd.tensor_mul(out=dxm[:], in0=dx[:], in1=htb[:])
                sq3 = wpool.tile([P, FI], bf16, tag="sq")
                nc.scalar.activation(sq3[:], dxm[:], SQUARE,
                                     accum_out=msum_t[:, 3 * g + 2:3 * g + 3])

            nc.sync.dma_start(out=msum_d, in_=msum_t[:])
            nc.sync.dma_start(out=vals_d, in_=vals_t[:])
            nc.sync.dma_start(out=idx_d, in_=idx_t[:])

    nc.compile()
    return nc


def _get_nc():
    if "nc" not in _STATE:
        _STATE["nc"] = _build()
    return _STATE["nc"]


def _softplus(x):
    return np.log1p(np.exp(-np.abs(x))) + np.maximum(x, 0.0)


def run_device(in_maps, **kwargs):
    from concourse.bass_utils import run_bass_kernel_spmd
    nc = _get_nc()
    return run_bass_kernel_spmd(nc, in_maps, core_ids=list(range(NCORES)), **kwargs)


def make_in_maps(inp):
    src = {"hp": inp["heat_predictions"], "ht": inp["heat_targets"],
           "m": inp["masks"], "oyp": inp["offy_predictions"],
           "oyt": inp["offy_targets"], "oxp": inp["offx_predictions"],
           "oxt": inp["offx_targets"]}
    full = {}
    for name, arr in src.items():
        a = np.ascontiguousarray(arr, dtype=np.float32).reshape(B * C, PI, FI)
        if name != "ht":
            a = a.astype(ml_dtypes.bfloat16)
        full[name] = a.reshape(B * C // NPAIR * NGROUP, P, FI)
    return [{name: arr[k * NGROUP:(k + 1) * NGROUP] for name, arr in full.items()}
            for k in range(NCORES)]


def finish_host(results, inp):
    """Combine per-core device outputs into the final scalar loss (float64 host math)."""
    cp = np.asarray(inp["clss_predictions"], dtype=np.float32).reshape(B, C, H * W)
    ct = np.asarray(inp["clss_targets"], dtype=np.float32).reshape(B, C, H * W)
    v_w = float(np.asarray(inp["v_loss_weight"]))
    d_w = float(np.asarray(inp["d_loss_weight"]))

    ssq = np.zeros(3, dtype=np.float64)
    g_pred = np.zeros((B, C), dtype=np.float64)
    g_tgt = np.zeros((B, C), dtype=np.float64)
    for k in range(NCORES):
        out = results[k]
        mm = np.asarray(out["msum"], dtype=np.float64).reshape(P, NGROUP, 3)
        ssq += mm.sum(axis=(0, 1))
        pm = np.asarray(out["vals8"]).reshape(P, NGROUP, 8)[:, :, 0]
        ji = np.asarray(out["idx8"]).reshape(P, NGROUP, 8)[:, :, 0]
        for g in range(NGROUP):
            for h in range(T):
                i = T * g + h          # image index within core
                b = k * BPC + i // C
                c = i % C
                rows = slice(PI * h, PI * h + PI)
                p_star = int(np.argmax(pm[rows, g]))  # first max part == lowest flat
                flat = p_star * FI + int(ji[PI * h + p_star, g])
                g_pred[b, c] = cp[b, c, flat]
                g_tgt[b, c] = ct[b, c, flat]

    n_el = float(B * C * H * W)
    heat_loss = ssq[0] / n_el
    offy_loss = ssq[1] / n_el
    offx_loss = ssq[2] / n_el

    valid = g_tgt >= 0.0
    is_v = (np.arange(C) < N_V_CHANNELS)[None, :]
    v_mask = (valid & is_v).astype(np.float64)
    d_mask = (valid & ~is_v).astype(np.float64)

    x = g_pred
    sp_neg = _softplus(-x)
    sp_pos = _softplus(x)

    l_v = POS_W_V * g_tgt * sp_neg + (1.0 - g_tgt) * sp_pos
    v_cls = (l_v * v_mask).sum() / max(v_mask.sum(), 1.0)
    y_d = (g_tgt >= 1.0).astype(np.float64)
    l_d = POS_W_D * y_d * sp_neg + (1.0 - y_d) * sp_pos
    d_cls = (l_d * d_mask).sum() / max(d_mask.sum(), 1.0)

    loss = (heat_loss * HEAT_WEIGHT
            + offy_loss * OFFSET_WEIGHT + offx_loss * OFFSET_WEIGHT
            + v_cls * v_w + d_cls * d_w)
    return np.float32(loss)


def kernel(**inputs):
    inp = {k: np.asarray(v) for k, v in inputs.items()}
    in_maps = make_in_maps(inp)
    res = run_device(in_maps)
    return finish_host(res.results, inp)


# revision 4
# speedup vs baseline: 1.5613x; 1.4954x over previous
"""Trainium2 Bass kernel for nn_CCFLoss (masked-MSE heat/offset losses + argmax-gathered
class-balanced BCE), data-parallel over batch across 8 NeuronCores.

v4: the three masked-MSE sums are computed as Frobenius inner products
    sum((p-t)*w)^2 = <d^2, w^2> = trace((d^2)^T (w^2))
so the DVE only does subtractions (plus the exact f32 argmax scan), the ACT
engine does the elementwise squares, and the otherwise-idle TensorE contracts
everything into a single accumulating [128,128] PSUM bank whose diagonal the
host sums. GPSIMD is left idle on purpose: it shares an SBUF port with the
DVE, and any GPSIMD op measurably stretches concurrent DVE ops ~4x.

HBM traffic: 14 B/elem (ht f32 for the exact argmax tie-break, heat_pred+mask
as fp8e3 - they are only read by 1x-rate ops whose cost is dtype-independent -
offsets as bf16), vs 28 B/elem for the all-f32 baseline.

Device per core (2 batches = 22 images as [64, 1024] blocks; 11 groups of 2
images stacked on partitions -> [128, 1024] tiles; hp|m and oyp|oxp, oyt|oxt
are concatenated by the host into single DMA blocks):
  - DVE: max8 + max_index on f32 ht; dh = hp - ht; dyx = oyxp - oyxt; dh^2.
  - ACT: ht^2, m^2, dyx^2 (Square, bf16 out).
  - PE : 24 matmuls/group accumulate diag contributions into PSUM.
  - outputs per core: diag [128,128] f32, vals8 [128, 88] f32, idx8 [128, 88] u32.
Host: sums diag / n_el for the MSE part, picks the global argmax per (b,c)
from per-partition top-1s, gathers clss_* at those 176 locations, finishes the
masked BCE means on scalars in float64.
"""
import sys

if "/opt/trn_rl_repo" not in sys.path:
    sys.path.insert(0, "/opt/trn_rl_repo")

import numpy as np
import ml_dtypes

B, C, H, W = 16, 11, 256, 256
P = 128
PI, FI = 64, 1024          # one [H,W] image = [64, 1024] block
T = 2                      # images per group tile (stacked on partitions)
NCORES = 8
BPC = B // NCORES          # batches per core
NPAIR = BPC * C            # images per core (22)
NGROUP = NPAIR // T        # group tiles per core (11)
NCHUNK = FI // 128         # 128-col chunks per [128, FI] tile
N_V_CHANNELS = 5
HEAT_WEIGHT = 1.0
OFFSET_WEIGHT = 1.0

_STATE = {}


def _pos_weight(samples):
    s = np.asarray(samples, dtype=np.float64)
    beta = (s - 1.0) / s
    en = (1.0 - np.power(beta, s)) / (1.0 - beta)
    w = 1.0 / (en + 1e-5)
    return float(w[1] / (w[0] + 1e-5))


POS_W_V = _pos_weight([8000.0, 2000.0])
POS_W_D = _pos_weight([7000.0, 2000.0 + 1000.0])


def _build():
    import concourse.bacc as bacc
    import concourse.tile as tile
    import concourse.mybir as mybir

    f32 = mybir.dt.float32
    bf16 = mybir.dt.bfloat16
    fp8 = mybir.dt.float8e3
    u32 = mybir.dt.uint32
    SQUARE = mybir.ActivationFunctionType.Square

    nc = bacc.Bacc("TRN2", target_bir_lowering=False, debug=False)
    ins = {
        "ht": nc.dram_tensor("ht", [NGROUP, P, FI], f32, kind="ExternalInput").ap(),
        "hpm": nc.dram_tensor("hpm", [NGROUP, P, 2 * FI], fp8,
                              kind="ExternalInput").ap(),
        "oyxp": nc.dram_tensor("oyxp", [NGROUP, P, 2 * FI], bf16,
                               kind="ExternalInput").ap(),
        "oyxt": nc.dram_tensor("oyxt", [NGROUP, P, 2 * FI], bf16,
                               kind="ExternalInput").ap(),
    }
    diag_d = nc.dram_tensor("diag", [P, P], f32, kind="ExternalOutput").ap()
    vals_d = nc.dram_tensor("vals8", [P, 8 * NGROUP], f32, kind="ExternalOutput").ap()
    idx_d = nc.dram_tensor("idx8", [P, 8 * NGROUP], u32, kind="ExternalOutput").ap()

    n_mm = NGROUP * 3 * NCHUNK  # total accumulating matmuls
    with tile.TileContext(nc) as tc:
        with tc.tile_pool(name="ins", bufs=3) as ipool, \
             tc.tile_pool(name="work", bufs=3) as wpool, \
             tc.tile_pool(name="acc", bufs=1) as apool, \
             tc.tile_pool(name="ps", bufs=1, space="PSUM") as pspool:
            vals_t = apool.tile([P, 8 * NGROUP], f32)
            idx_t = apool.tile([P, 8 * NGROUP], u32)
            psum_t = pspool.tile([P, P], f32)

            mm_i = 0
            for g in range(NGROUP):
                t = {}
                for j, (name, dt, w) in enumerate((
                        ("ht", f32, FI), ("hpm", fp8, 2 * FI),
                        ("oyxp", bf16, 2 * FI), ("oyxt", bf16, 2 * FI))):
                    tt = ipool.tile([P, w], dt, tag=name)
                    # first group: co-issue from ACT so transfers start on
                    # whichever sequencer boots first
                    eng = nc.scalar if (g == 0 and j % 2 == 1) else nc.sync
                    eng.dma_start(out=tt[:], in_=ins[name][g])
                    t[name] = tt

                # per-partition top-8 of ht (f32, exact) - covers both images
                v8 = vals_t[:, 8 * g:8 * g + 8]
                nc.vector.max(out=v8, in_=t["ht"][:])
                nc.vector.max_index(out=idx_t[:, 8 * g:8 * g + 8],
                                    in_max=v8, in_values=t["ht"][:])

                # squares of the weights (ACT, 1x rate, any input dtype)
                ht2 = wpool.tile([P, FI], bf16, tag="ht2")
                nc.scalar.activation(ht2[:], t["ht"][:], SQUARE)
                m2 = wpool.tile([P, FI], bf16, tag="m2")
                nc.scalar.activation(m2[:], t["hpm"][:, FI:], SQUARE)

                # diffs (DVE) and their squares (heat on DVE, offsets on ACT)
                dh = wpool.tile([P, FI], bf16, tag="dh")
                nc.vector.tensor_sub(out=dh[:], in0=t["hpm"][:, :FI],
                                     in1=t["ht"][:])
                dh2 = wpool.tile([P, FI], bf16, tag="dh2")
                nc.vector.tensor_mul(out=dh2[:], in0=dh[:], in1=dh[:])
                dyx = wpool.tile([P, 2 * FI], bf16, tag="dyx")
                nc.vector.tensor_sub(out=dyx[:], in0=t["oyxp"][:],
                                     in1=t["oyxt"][:])
                dyx2 = wpool.tile([P, 2 * FI], bf16, tag="dyx2")
                nc.scalar.activation(dyx2[:], dyx[:], SQUARE)

                # accumulate sum(d^2 * w^2) = trace((d^2)^T (w^2)) chunkwise
                # into one PSUM bank; host reads the diagonal.
                for c in range(NCHUNK):
                    s = slice(128 * c, 128 * c + 128)
                    nc.tensor.matmul(psum_t[:], lhsT=dh2[:, s], rhs=m2[:, s],
                                     start=(mm_i == 0), stop=(mm_i == n_mm - 1))
                    mm_i += 1
                for c in range(2 * NCHUNK):
                    s = slice(128 * c, 128 * c + 128)
                    s2 = slice(128 * (c % NCHUNK), 128 * (c % NCHUNK) + 128)
                    nc.tensor.matmul(psum_t[:], lhsT=dyx2[:, s], rhs=ht2[:, s2],
                                     start=(mm_i == 0), stop=(mm_i == n_mm - 1))
                    mm_i += 1

            diag_s = apool.tile([P, P], f32)
            nc.scalar.copy(out=diag_s[:], in_=psum_t[:])
            nc.sync.dma_start(out=diag_d, in_=diag_s[:])
            nc.sync.dma_start(out=vals_d, in_=vals_t[:])
            nc.sync.dma_start(out=idx_d, in_=idx_t[:])

    nc.compile()
    return nc


def _get_nc():
    if "nc" not in _STATE:
        _STATE["nc"] = _build()
    return _STATE["nc"]


def _softplus(x):
    return np.log1p(np.exp(-np.abs(x))) + np.maximum(x, 0.0)


def run_device(in_maps, **kwargs):
    from concourse.bass_utils import run_bass_kernel_spmd
    nc = _get_nc()
    return run_bass_kernel_spmd(nc, in_maps, core_ids=list(range(NCORES)), **kwargs)


def _groups(a):
    """[B*C, H, W] f32 -> [B*C//NPAIR*NGROUP, P, FI] (pure reshape)."""
    return np.ascontiguousarray(a, dtype=np.float32).reshape(-1, P, FI)


def make_in_maps(inp):
    fp8 = ml_dtypes.float8_e3m4
    bf16 = ml_dtypes.bfloat16
    ht = _groups(inp["heat_targets"])
    hpm = np.concatenate([_groups(inp["heat_predictions"]).astype(fp8),
                          _groups(inp["masks"]).astype(fp8)], axis=2)
    oyxp = np.concatenate([_groups(inp["offy_predictions"]).astype(bf16),
                           _groups(inp["offx_predictions"]).astype(bf16)], axis=2)
    oyxt = np.concatenate([_groups(inp["offy_targets"]).astype(bf16),
                           _groups(inp["offx_targets"]).astype(bf16)], axis=2)
    full = {"ht": ht, "hpm": hpm, "oyxp": oyxp, "oyxt": oyxt}
    return [{name: arr[k * NGROUP:(k + 1) * NGROUP] for name, arr in full.items()}
            for k in range(NCORES)]


def finish_host(results, inp):
    """Combine per-core device outputs into the final scalar loss (float64 host math)."""
    cp = np.asarray(inp["clss_predictions"], dtype=np.float32).reshape(B, C, H * W)
    ct = np.asarray(inp["clss_targets"], dtype=np.float32).reshape(B, C, H * W)
    v_w = float(np.asarray(inp["v_loss_weight"]))
    d_w = float(np.asarray(inp["d_loss_weight"]))

    mse_sum = 0.0
    g_pred = np.zeros((B, C), dtype=np.float64)
    g_tgt = np.zeros((B, C), dtype=np.float64)
    for k in range(NCORES):
        out = results[k]
        mse_sum += float(np.trace(np.asarray(out["diag"], dtype=np.float64)))
        pm = np.asarray(out["vals8"]).reshape(P, NGROUP, 8)[:, :, 0]
        ji = np.asarray(out["idx8"]).reshape(P, NGROUP, 8)[:, :, 0]
        for g in range(NGROUP):
            for h in range(T):
                i = T * g + h          # image index within core
                b = k * BPC + i // C
                c = i % C
                rows = slice(PI * h, PI * h + PI)
                p_star = int(np.argmax(pm[rows, g]))  # first max part == lowest flat
                flat = p_star * FI + int(ji[PI * h + p_star, g])
                g_pred[b, c] = cp[b, c, flat]
                g_tgt[b, c] = ct[b, c, flat]

    n_el = float(B * C * H * W)
    mse_loss = mse_sum / n_el   # heat + offy + offx (all weights are 1.0)

    valid = g_tgt >= 0.0
    is_v = (np.arange(C) < N_V_CHANNELS)[None, :]
    v_mask = (valid & is_v).astype(np.float64)
    d_mask = (valid & ~is_v).astype(np.float64)

    x = g_pred
    sp_neg = _softplus(-x)
    sp_pos = _softplus(x)

    l_v = POS_W_V * g_tgt * sp_neg + (1.0 - g_tgt) * sp_pos
    v_cls = (l_v * v_mask).sum() / max(v_mask.sum(), 1.0)
    y_d = (g_tgt >= 1.0).astype(np.float64)
    l_d = POS_W_D * y_d * sp_neg + (1.0 - y_d) * sp_pos
    d_cls = (l_d * d_mask).sum() / max(d_mask.sum(), 1.0)

    loss = mse_loss + v_cls * v_w + d_cls * d_w
    return np.float32(loss)


def kernel(**inputs):
    inp = {k: np.asarray(v) for k, v in inputs.items()}
    in_maps = make_in_maps(inp)
    res = run_device(in_maps)
    return finish_host(res.results, inp)
